# revision 7
# baseline (speedup 1.0000x reference)
"""Multi-head causal self-attention (B=2, S=2048, E=1024, H=16, D=64) on 8 TRN2
NeuronCores.

Sharding: core c owns batch b = c//4 and head-group g = c%4 (4 heads each).
Per core, transpose-free layout:
  QT/KT [d_local=256, S]  (d on partitions),  V [S, d_local] (t on partitions),
  scoresT [t, s] blocks via lhsT=KT-block, softmax is unnormalized exp (scores
  ~N(0,1); no max subtraction needed in f32), sums ride free in a
  ones-augmented V ([V|1] -> M=65 AV matmul, row 64 = column sums), output
  projection is row-parallel; the host sums 4 bf16 partials per batch + bp.

Schedule: single software-pipelined stream.  The attention j-loop is the
backbone (ACT exp is the per-iteration bound); all other PE work (QKV
projections for later s-blocks, V tiles, output-projection units, the
normalization broadcast matmuls) is dripped into the loop as deferred units
so the PE queue never runs dry (keeps the PE at its max p-state).  AV
matmuls lag scores/exp by 2 iterations to absorb exp latency and block
boundary normalization. Score PSUM ping-pong (2x2 banks) + AV accumulators
(4 banks) fill PSUM; drip tiles (proj/broadcast/QKV accumulators)
time-share the score ring.  rsum copies + causal tri-mask multiplies run on
GPSIMD, recip/normalize/evacuations on DVE, exp + QKV bias-evac on ACT.
"""

import numpy as np
import ml_dtypes

import concourse.bass as bass
import concourse.tile as tile
from concourse import bacc, mybir
from concourse import bass_utils

B, S, E, H, D = 2, 2048, 1024, 16, 64
NCORES = 8
HPC = 4                 # heads per core
EL = HPC * D            # 256 local channels
SBW = 512               # s-block width
NSB = S // SBW          # 4
TBW = 128               # t-block width
NTB = S // TBW          # 16
NEB = E // 128          # 8 e-blocks
SCALE = 1.0 / np.sqrt(D)

F32 = mybir.dt.float32
BF16 = mybir.dt.bfloat16

_BUILT = None


def _emit(tc, nc, d):
    Exp = mybir.ActivationFunctionType.Exp
    Ident = mybir.ActivationFunctionType.Identity

    with (
        tc.tile_pool(name="const", bufs=1) as cst,
        tc.tile_pool(name="big", bufs=1) as big,
        tc.tile_pool(name="ptp", bufs=8) as ptp,
        tc.tile_pool(name="rsp", bufs=4) as rsp,
        tc.tile_pool(name="bcsp", bufs=4) as bcsp,
        tc.tile_pool(name="outp", bufs=4) as outp,
        tc.tile_pool(name="scp", bufs=2, space="PSUM") as scp,
        tc.tile_pool(name="avsp", bufs=4, space="PSUM") as avsp,
    ):
        # ---- inputs: few big DMAs, split across SP and ACT queues ----
        xt = big.tile([128, NEB * S], BF16, name="xt", tag="xt")
        xt3 = xt.rearrange("p (e s) -> p e s", s=S)
        xtd3 = d["xt"].rearrange("p (e s) -> p e s", s=S)
        nc.sync.dma_start(xt3[:, :, 0:SBW], xtd3[:, :, 0:SBW])

        wq = big.tile([128, NEB * EL], BF16, name="wq", tag="wq")
        wq3 = wq.rearrange("p (e t c) -> p e t c", t=2, c=128)
        wqd3 = d["wq"].rearrange("p (e t c) -> p e t c", t=2, c=128)
        nc.scalar.dma_start(wq3[:, :, 0:1], wqd3[:, :, 0:1])
        bq = cst.tile([128, 2], F32, name="bq", tag="bq")
        nc.scalar.dma_start(bq[:], d["bq"][:])
        nc.scalar.dma_start(wq3[:, :, 1:2], wqd3[:, :, 1:2])
        wk = big.tile([128, NEB * EL], BF16, name="wk", tag="wk")
        nc.scalar.dma_start(wk[:], d["wk"][:])
        bk = cst.tile([128, 2], F32, name="bk", tag="bk")
        nc.scalar.dma_start(bk[:], d["bk"][:])
        wv = big.tile([128, NEB * EL], BF16, name="wv", tag="wv")
        nc.scalar.dma_start(wv[:], d["wv"][:])
        bv = cst.tile([128, EL], F32, name="bv", tag="bv")
        nc.scalar.dma_start(bv[:], d["bv"][:])
        for i in range(1, NSB):
            nc.sync.dma_start(
                xt3[:, :, i * SBW:(i + 1) * SBW], xtd3[:, :, i * SBW:(i + 1) * SBW]
            )
        wp = big.tile([128, 2 * E], BF16, name="wp", tag="wp")
        nc.scalar.dma_start(wp[:], d["wp"][:])
        tri = cst.tile([128, 128], BF16, name="tri", tag="tri")
        nc.scalar.dma_start(tri[:], d["tri"][:])
        ones = cst.tile([128, 64], BF16, name="ones", tag="ones")
        nc.vector.memset(ones[:], 1.0)

        # V tiles [128, 4*65]: head h at cols 65h..65h+64, ones col at 65h+64
        vt = []
        for j in range(NTB):
            t = big.tile([128, HPC * 65], BF16, name=f"vt{j}", tag=f"vt{j}")
            nc.vector.memset(
                t.rearrange("p (h c) -> p h c", c=65)[:, :, 64:65], 1.0
            )
            vt.append(t)

        qt = [big.tile([128, S], BF16, name=f"qt{k}", tag=f"qt{k}")
              for k in range(2)]
        kt = [big.tile([128, S], BF16, name=f"kt{k}", tag=f"kt{k}")
              for k in range(2)]
        yt = [big.tile([128, S], BF16, name=f"yt{k}", tag=f"yt{k}")
              for k in range(2)]

        # ---- deferred work units (dripped into the attention j-loop) ----
        def emit_qk_unit(ic, dst, wl, bl, dt_i):
            ac = scp.tile([128, SBW], F32, name="qk_ac", tag="sc")
            for e in range(NEB):
                nc.tensor.matmul(
                    ac[:],
                    wl[:, e * EL + dt_i * 128: e * EL + dt_i * 128 + 128],
                    xt[:, e * S + ic * SBW: e * S + (ic + 1) * SBW],
                    start=(e == 0),
                    stop=(e == NEB - 1),
                )
            nc.scalar.activation(
                dst[dt_i][:, ic * SBW:(ic + 1) * SBW], ac[:], Ident,
                bias=bl[:, dt_i:dt_i + 1], scale=1.0,
            )

        def emit_v_unit(j16):
            ac = scp.tile([128, EL], F32, name="v_ac", tag="sc")
            for e in range(NEB):
                nc.tensor.matmul(
                    ac[:],
                    xt[:, e * S + j16 * TBW: e * S + (j16 + 1) * TBW],
                    wv[:, e * EL:(e + 1) * EL],
                    start=(e == 0),
                    stop=(e == NEB - 1),
                )
            nc.vector.tensor_add(
                vt[j16].rearrange("p (h c) -> p h c", c=65)[:, :, 0:64],
                ac.rearrange("p (h c) -> p h c", c=64),
                bv.rearrange("p (h c) -> p h c", c=64),
            )

        def emit_proj_unit(r0, nb2):
            pr = scp.tile([128, 512], F32, name="pr", tag="sc")
            for cb in range(2):
                nc.tensor.matmul(
                    pr[:],
                    yt[cb][:, r0:r0 + 128],
                    wp[:, cb * E + nb2 * 512: cb * E + (nb2 + 1) * 512],
                    start=(cb == 0),
                    stop=(cb == 1),
                )
            ot = outp.tile([128, 512], BF16, name="ot", tag="ot")
            nc.vector.tensor_copy(ot[:], pr[:])
            nc.sync.dma_start(
                d["out"][r0:r0 + 128, nb2 * 512:(nb2 + 1) * 512], ot[:]
            )

        def emit_norm_head(i, h, avs_i):
            # yt[h//2][64*(h%2)+.., s-block i] = av[0:64] / av[64]
            dt_i, po = h // 2, 64 * (h % 2)
            rsum = rsp.tile([65, SBW], BF16, name="rsum", tag="rs")
            nc.vector.tensor_copy(rsum[64:65, :], avs_i[h][64:65, :])
            bc = scp.tile([64, SBW], F32, name="bc", tag="sc")
            nc.tensor.matmul(
                bc[:], ones[64:65, 0:64], rsum[64:65, :],
                start=True, stop=True,
            )
            bcr = bcsp.tile([64, SBW], F32, name="bcr", tag="bcs")
            nc.vector.reciprocal_approx_fast(bcr[:], bc[:])
            nc.vector.tensor_mul(
                yt[dt_i][po:po + 64, i * SBW:(i + 1) * SBW],
                avs_i[h][0:64, :],
                bcr[:],
            )

        deferred = []       # (deadline_key, est_pe_ns, closure)
        END = (NSB, 99)

        def pump(now, budget_ns):
            # overdue units first (correctness: producers must be emitted
            # before their consumers), then budget-limited extras
            k = 0
            while k < len(deferred):
                if deferred[k][0] <= now:
                    deferred.pop(k)[2]()
                else:
                    k += 1
            spent = 0
            while deferred and spent < budget_ns:
                _, est, fn = deferred.pop(0)
                fn()
                spent += est

        QK_NS, V_NS, PROJ_NS = 1800, 900, 700

        # ---- preamble: qt/kt chunk 0, V t-blocks 0-3 ----
        for dst, wl, bl in ((qt, wq, bq), (kt, wk, bk)):
            for dt_i in range(2):
                emit_qk_unit(0, dst, wl, bl, dt_i)
        for j16 in range(4):
            emit_v_unit(j16)
        for dt_i in range(2):
            deferred.append(
                ((1, -1), QK_NS, lambda t=dt_i: emit_qk_unit(1, qt, wq, bq, t)))

        # ---- attention: software-pipelined s-block loop ----
        for i in range(NSB):
            pump((i, -1), 0)   # drain units this block needs up front
            avs = [avsp.tile([65, SBW], F32, name=f"av{h}", tag="av")
                   for h in range(HPC)]
            njs = 4 * i + 4

            def av_mms(pts_, w_, j_, avs_=None, njs_=None):
                a = avs if avs_ is None else avs_
                n = njs if njs_ is None else njs_
                for h in range(HPC):
                    nc.tensor.matmul(
                        a[h][:, w_:SBW],
                        vt[j_][:, 65 * h: 65 * h + 65],
                        pts_[h // 2][:, (h % 2) * SBW + w_:
                                     (h % 2 + 1) * SBW],
                        start=(j_ == 0),
                        stop=(j_ == n - 1),
                    )

            pending = []  # AV deferred by 2 iterations
            for j in range(njs):
                w = 128 * (j - 4 * i) if j >= 4 * i else 0  # skipped cols
                cw = SBW - w                                # computed width
                pts = []
                for p in range(2):  # head pairs (0,1) and (2,3)
                    sc2 = scp.tile([128, 2 * SBW], F32, name="sc2", tag="sc")
                    for hh in range(2):
                        h = 2 * p + hh
                        dt_i, po = h // 2, 64 * (h % 2)
                        nc.tensor.matmul(
                            sc2[:, hh * SBW: hh * SBW + cw],
                            kt[dt_i][po:po + 64, j * TBW:(j + 1) * TBW],
                            qt[dt_i][po:po + 64,
                                     i * SBW + w: (i + 1) * SBW],
                            start=True,
                            stop=True,
                        )
                    pt_t = ptp.tile([128, 2 * SBW], BF16, name="ptile",
                                    tag="pt")
                    nc.scalar.activation(
                        pt_t.rearrange("q (g c) -> q g c", c=SBW)[:, :, w:SBW],
                        sc2.rearrange("q (g c) -> q g c", c=SBW)[:, :, 0:cw],
                        Exp,
                    )
                    if j >= 4 * i:  # diagonal: 0/1 triangular mask on PT
                        for hh in range(2):
                            zone = hh * SBW + w
                            nc.gpsimd.tensor_mul(
                                pt_t[:, zone: zone + 128],
                                pt_t[:, zone: zone + 128],
                                tri[:],
                            )
                    pts.append(pt_t)
                pending.append((pts, w, j))
                if len(pending) > 2:
                    av_mms(*pending.pop(0))
                pump((i, j), 1000 if j % 2 == 0 else 700)
            for u in pending:
                av_mms(*u)

            # normalization must be emitted before block i+1's first AV
            for h in range(HPC):
                emit_norm_head(i, h, avs)

            # ---- queue next block's inputs + this block's projection ----
            if i < NSB - 1:
                for dt_i in range(2):
                    deferred.append(
                        ((i + 1, 4 * (i + 1) - 2), QK_NS,
                         lambda t=dt_i, c=i + 1:
                         emit_qk_unit(c, kt, wk, bk, t)))
                for j16 in range(4 * (i + 1), 4 * (i + 1) + 4):
                    deferred.append(
                        ((i + 1, j16), V_NS, lambda jj=j16: emit_v_unit(jj)))
                if i + 2 <= NSB - 1:
                    for dt_i in range(2):
                        deferred.append(
                            ((i + 2, -1), QK_NS,
                             lambda t=dt_i, c=i + 2:
                             emit_qk_unit(c, qt, wq, bq, t)))
            for st in range(4):
                for nb2 in range(2):
                    deferred.append(
                        (END, PROJ_NS,
                         lambda r=i * SBW + st * 128, n=nb2:
                         emit_proj_unit(r, n)))

        # ---- epilogue: drain remaining deferred work ----
        while deferred:
            deferred.pop(0)[2]()


def _build():
    global _BUILT
    if _BUILT is not None:
        return _BUILT
    nc = bacc.Bacc("TRN2", target_bir_lowering=False, debug=False,
                   num_devices=NCORES)
    d = {
        "xt": nc.dram_tensor("xt", [128, NEB * S], BF16, kind="ExternalInput").ap(),
        "wq": nc.dram_tensor("wq", [128, NEB * EL], BF16, kind="ExternalInput").ap(),
        "wk": nc.dram_tensor("wk", [128, NEB * EL], BF16, kind="ExternalInput").ap(),
        "wv": nc.dram_tensor("wv", [128, NEB * EL], BF16, kind="ExternalInput").ap(),
        "wp": nc.dram_tensor("wp", [128, 2 * E], BF16, kind="ExternalInput").ap(),
        "bq": nc.dram_tensor("bq", [128, 2], F32, kind="ExternalInput").ap(),
        "bk": nc.dram_tensor("bk", [128, 2], F32, kind="ExternalInput").ap(),
        "bv": nc.dram_tensor("bv", [128, EL], F32, kind="ExternalInput").ap(),
        "tri": nc.dram_tensor("tri", [128, 128], BF16, kind="ExternalInput").ap(),
        "out": nc.dram_tensor("out", [S, E], BF16, kind="ExternalOutput").ap(),
    }
    with tile.TileContext(nc) as tc:
        _emit(tc, nc, d)
    nc.compile()
    _BUILT = nc
    return _BUILT


def _blockify(a, pblk):
    """[N*pblk, M] -> [pblk, N*M] with block-column layout."""
    n = a.shape[0] // pblk
    return np.ascontiguousarray(
        a.reshape(n, pblk, a.shape[1]).transpose(1, 0, 2).reshape(pblk, -1)
    )


def _prep_core(c, x, Wq, bq, Wk, bk, Wv, bv, Wp):
    b, g = c // 4, c % 4
    lo = EL * g
    bf = ml_dtypes.bfloat16

    xT = np.ascontiguousarray(x[b].T)                        # [E, S]
    wqT = np.ascontiguousarray(Wq[lo:lo + EL, :].T) * SCALE  # [E, 256]
    wkT = np.ascontiguousarray(Wk[lo:lo + EL, :].T)
    wvT = np.ascontiguousarray(Wv[lo:lo + EL, :].T)
    wpT = np.ascontiguousarray(Wp[:, lo:lo + EL].T)          # [256, E]

    col = np.arange(128, dtype=np.int64)
    tri = np.where(col[None, :] >= np.arange(128)[:, None], 1.0, 0.0)

    return {
        "xt": _blockify(xT, 128).astype(bf),
        "wq": _blockify(wqT, 128).astype(bf),
        "wk": _blockify(wkT, 128).astype(bf),
        "wv": _blockify(wvT, 128).astype(bf),
        "wp": _blockify(wpT, 128).astype(bf),
        "bq": np.ascontiguousarray(
            (bq[lo:lo + EL] * SCALE).reshape(2, 128).T).astype(np.float32),
        "bk": np.ascontiguousarray(
            bk[lo:lo + EL].reshape(2, 128).T).astype(np.float32),
        "bv": np.ascontiguousarray(
            np.broadcast_to(bv[lo:lo + EL], (128, EL))).astype(np.float32),
        "tri": tri.astype(bf),
    }


def run(inputs, trace=False):
    """Run on hardware. Returns (out [B,S,E] f32, exec_time_ns or None)."""
    x = np.asarray(inputs["x"], np.float32)
    Wq = np.asarray(inputs["Wq"], np.float32)
    bq = np.asarray(inputs["bq"], np.float32)
    Wk = np.asarray(inputs["Wk"], np.float32)
    bk = np.asarray(inputs["bk"], np.float32)
    Wv = np.asarray(inputs["Wv"], np.float32)
    bv = np.asarray(inputs["bv"], np.float32)
    Wp = np.asarray(inputs["Wp"], np.float32)
    bp = np.asarray(inputs["bp"], np.float32)

    nc = _build()
    in_maps = [
        _prep_core(c, x, Wq, bq, Wk, bk, Wv, bv, Wp) for c in range(NCORES)
    ]
    kwargs = {}
    if trace:
        try:
            import ntff_shim
            ntff_shim.install()
        except Exception:
            pass
        kwargs["trace"] = True
    res = bass_utils.run_bass_kernel_spmd(
        nc, in_maps, list(range(NCORES)), **kwargs
    )
    out = np.empty((B, S, E), np.float32)
    for b in range(B):
        acc = res.results[4 * b]["out"].astype(np.float32)
        for g in range(1, 4):
            acc += res.results[4 * b + g]["out"].astype(np.float32)
        out[b] = acc + bp[None, :]
    return out, res.exec_time_ns


def kernel(**inputs):
    out, _ = run(inputs, trace=False)
    return out
